# revision 23
# baseline (speedup 1.0000x reference)
"""TRN2 Bass kernel for nn_DiscreteComms (VQ codebook assignment).

Computation (per reference):
  logits L = x @ W + b                      (32768 tokens x 512)
  flat = L viewed as (262144 rows x 64), rows r = token*8 + comm
  idx_r = argmin_v ||flat_r - cb_v||^2      (V=512)
  comm_output = cb[idx]                      (straight-through forward = q)
  vq_loss = 1.25 * mean((cb[idx] - flat)^2)
  comm_log_probs = zeros

Key identities used:
  argmin_v d = argmax_v (2*flat.cb_v - ||cb_v||^2)  =: argmax_v w
  ||q - flat||^2 per row = ||flat||^2 - max_v w
  => vq_loss needs only sum(L^2) and sum of per-row max scores; the gather
     feeds only comm_output (done by DMA engines).

Sharding: data-parallel over batch B across 8 NeuronCores (512 batch rows,
4096 tokens, 32768 VQ rows per core). W/b/codebook replicated.

Per-core pipeline:
  PE : transpose X -> X^T; GEMM1 (fp32); scores S=2g via K=64 pair-packed
       fp32 matmuls (tile_position row packing)
  DVE: two custom fused ops per score tile reading S from PSUM with the
       -||cb||^2 correction as a second stream:
         ARGMAX: w=S+ncsq; running-max scan; accum_out = argmax index
         MAXW  : accum_out = max w  (for the loss)
  ACT: PSUM->SBUF copies (X^T, L with bias b), sum(L^2) via Square+accum
  DMA: dma_gather of codebook rows by idx (wrapped int16 index list),
       2KB-burst writeback to the output layout
"""

import numpy as np

import concourse.bass as bass
import concourse.mybir as mybir
import concourse.tile as tile
from concourse import bacc
from concourse import bass_utils
from concourse import dve_ops
from concourse.dve_ops import DveOp, OPS
from concourse.dve_spec import (
    Spec, Src0, Src1, Zero, One, MaxNeg, Idx, scan, select, eq, lower, AluOp,
)
from concourse.dve_uop import DveOpSpec
from concourse.masks import make_identity

F32 = mybir.dt.float32
F16 = mybir.dt.float16
I16 = mybir.dt.int16

# problem constants (hardcoded; kernel.py must be self-contained)
B, N, H = 4096, 8, 512
NCOMM, C, V = 8, 64, 512
NCORES = 8
TOK = B * N // NCORES            # 4096 tokens per core
ROWS = TOK * NCOMM               # 32768 VQ rows per core
BLOCKS = 8                       # token blocks of 512 per core
TSUB = 4                         # 128-token subtiles per block
NT = BLOCKS * TSUB               # 32 token subtiles per core
NJ = NCOMM * NT                  # 256 (c, t) score tiles per core
GCHUNK = 1024                    # idx per dma_gather call (>=2048 crashes the DGE)
NCHUNK = ROWS // GCHUNK


def _register_op(name, spec):
    for op in OPS:
        if op.name == name:
            return op
    shas = {}
    for ver in ("v3", "v4"):
        s = DveOpSpec(name=name, opcode=0, uops=lower(spec, ver=ver), rd1_en=True)
        shas[ver] = s.sha(ver)
    op = DveOp(name, spec, subdim=False, uops_sha=shas)
    OPS.append(op)
    dve_ops.CUSTOM_DVE_SPECS[op.name] = op.spec
    dve_ops._SUB_OPCODE_FOR_NAME[op.name] = (
        dve_ops._CUSTOM_DVE_ROW_BASE + OPS.index(op)
    )
    assert dve_ops._SUB_OPCODE_FOR_NAME[op.name] < 0x20, "OPS row overflow"
    return op


def _make_ops():
    w = Src0 + Src1
    r = scan(AluOp.MAX, w, init=MaxNeg)
    argmax_op = _register_op(
        "VQ_ARGMAX_ANT",
        Spec(
            body=select(eq(w, r), Idx, Zero - One),
            accum=AluOp.MAX,
            accum_init=MaxNeg,
            reference=lambda in0, in1, s0, s1, imm2: (
                lambda body: (body, body.max(axis=-1, keepdims=True))
            )(
                np.where(
                    (in0 + in1)
                    == np.maximum.accumulate(in0 + in1, axis=-1),
                    np.arange(in0.shape[-1], dtype=np.float32),
                    -1.0,
                )
            ),
        ),
    )
    w2 = Src0 + Src1
    maxw_op = _register_op(
        "VQ_MAXW_ANT",
        Spec(
            body=w2,
            accum=AluOp.MAX,
            accum_init=MaxNeg,
            reference=lambda in0, in1, s0, s1, imm2: (
                in0 + in1,
                (in0 + in1).max(axis=-1, keepdims=True),
            ),
        ),
    )
    return argmax_op, maxw_op


def build_kernel(do_gather=True, gchunk=GCHUNK, flat_wb=False, wb='fancy', maxch=None, skip_maxw=False, skip_arg=False):
    argmax_op, maxw_op = _make_ops()
    nc = bacc.Bacc("TRN2", target_bir_lowering=False, debug=False,
                   enable_asserts=True)

    x_d = nc.dram_tensor("x_d", [TOK, H], F32, kind="ExternalInput")
    w_d = nc.dram_tensor("w_d", [H, H], F32, kind="ExternalInput")
    b_d = nc.dram_tensor("b_d", [1, H], F32, kind="ExternalInput")
    cb_d = nc.dram_tensor("cb_d", [V, C], F32, kind="ExternalInput")
    q_d = nc.dram_tensor("q_d", [ROWS, C], F32, kind="ExternalOutput")
    loss_d = nc.dram_tensor("loss_d", [1, 1], F32, kind="ExternalOutput")

    with tile.TileContext(nc) as tc:
        with (
            tc.tile_pool(name="const", bufs=1) as constp,
            tc.tile_pool(name="xin", bufs=3) as xinp,
            tc.tile_pool(name="xtp", bufs=2) as xtp,
            tc.tile_pool(name="lsb", bufs=2) as lsbp,
            tc.tile_pool(name="junk", bufs=8) as junkp,
            tc.tile_pool(name="qout", bufs=2) as qoutp,
            tc.tile_pool(name="ps", bufs=2, space="PSUM") as psp,
            tc.tile_pool(name="pss", bufs=2, space="PSUM") as pssp,
            tc.tile_pool(name="ps1", bufs=1, space="PSUM") as ps1p,
        ):
            # ---- constants --------------------------------------------------
            ident = constp.tile([128, 128], F32)
            make_identity(nc, ident)

            w_sb = constp.tile([128, 4 * H], F32)  # [kb*H : kb*H+H] = W[128kb:,:]
            for kb in range(4):
                nc.sync.dma_start(
                    w_sb[:, kb * H : (kb + 1) * H], w_d.ap()[128 * kb : 128 * (kb + 1), :]
                )
            b_sb = constp.tile([128, 4], F32)  # b[128*mb + p] at [p, mb]
            nc.sync.dma_start(
                b_sb[:, :], b_d.ap().rearrange("o (mb p) -> (o p) mb", mb=4)
            )

            cb_nat = constp.tile([128, 4 * C], F32)  # 4 vblocks of cb rows
            for vb in range(4):
                nc.sync.dma_start(
                    cb_nat[:, vb * C : (vb + 1) * C],
                    cb_d.ap()[128 * vb : 128 * (vb + 1), :],
                )
            negones = constp.tile([C, 128], F32)
            nc.gpsimd.memset(negones, -1.0)

            movT = constp.tile([128, V], F32)    # 2*CB^T duplicated in halves
            cbsqT = constp.tile([C, V], F32)     # (CB^T)^2
            ncsq_sb = constp.tile([128, V], F32)  # -||cb_v||^2 bcast over parts
            for vb in range(4):
                tp = ps1p.tile([C, 128], F32, tag="one")
                nc.tensor.transpose(tp, cb_nat[:, vb * C : (vb + 1) * C], ident)
                sl = slice(128 * vb, 128 * (vb + 1))
                nc.scalar.mul(movT[0:C, sl], tp, 2.0)
                nc.scalar.mul(movT[C:128, sl], tp, 2.0)
                nc.scalar.square(cbsqT[:, sl], tp)
            ncsq_ps = ps1p.tile([128, V], F32, tag="one")
            nc.tensor.matmul(ncsq_ps, negones, cbsqT, start=True, stop=True)
            nc.scalar.copy(ncsq_sb, ncsq_ps)

            # accumulators
            m_buf = constp.tile([128, NJ], F32)
            idxf_buf = constp.tile([128, NJ], F32)
            l2_buf = constp.tile([128, 4 * BLOCKS], F32)
            idx16 = constp.tile([128, NJ], I16)
            idxw = constp.tile([128, 8 * NJ], I16)

            # ---- main per-block pipeline -----------------------------------
            for blk in range(BLOCKS):
                # load X natural, transpose to X^T
                xt_sb = xtp.tile([128, 4 * 512], F32, tag="xt")  # kb-major
                for ts in range(TSUB):
                    xn = xinp.tile([128, H], F32, tag="xn")
                    nc.sync.dma_start(
                        xn, x_d.ap()[blk * 512 + ts * 128 : blk * 512 + (ts + 1) * 128, :]
                    )
                    xt_ps = psp.tile([128, 512], F32, tag="blkps")
                    for kb in range(4):
                        nc.tensor.transpose(
                            xt_ps[:, kb * 128 : (kb + 1) * 128],
                            xn[:, kb * 128 : (kb + 1) * 128],
                            ident,
                        )
                    for kb in range(4):
                        nc.scalar.copy(
                            xt_sb[:, kb * 512 + ts * 128 : kb * 512 + (ts + 1) * 128],
                            xt_ps[:, kb * 128 : (kb + 1) * 128],
                        )

                # GEMM1: L^T chunks (out-dims on partitions)
                l_sb = lsbp.tile([128, 4 * 512], F32, tag="lsb")  # mb-major
                for mb in range(4):
                    l_ps = psp.tile([128, 512], F32, tag="blkps")
                    for kb in range(4):
                        nc.tensor.matmul(
                            l_ps,
                            w_sb[:, kb * H + 128 * mb : kb * H + 128 * (mb + 1)],
                            xt_sb[:, kb * 512 : (kb + 1) * 512],
                            start=(kb == 0),
                            stop=(kb == 3),
                        )
                    lslice = l_sb[:, mb * 512 : (mb + 1) * 512]
                    nc.scalar.activation(
                        lslice, l_ps, mybir.ActivationFunctionType.Identity,
                        bias=b_sb[:, mb : mb + 1], scale=1.0,
                    )
                    l2junk = junkp.tile([128, 512], F16, tag="l2junk")
                    nc.scalar.activation(
                        l2junk, lslice, mybir.ActivationFunctionType.Square,
                        accum_out=l2_buf[:, blk * 4 + mb : blk * 4 + mb + 1],
                    )

                # scores + fused argmax/max per (comm, token-subtile)
                for mb in range(4):
                    for ts in range(TSUB):
                        t_glob = blk * TSUB + ts
                        sA = pssp.tile([128, V], F32, tag="sA")
                        sB = pssp.tile([128, V], F32, tag="sB")
                        lt = l_sb[:, mb * 512 + ts * 128 : mb * 512 + (ts + 1) * 128]
                        nc.tensor.matmul(
                            sA, lt[0:C, :], movT[0:C, :], start=True, stop=True
                        )
                        nc.tensor.matmul(
                            sB, lt[C:128, :], movT[C:128, :], start=True, stop=True
                        )
                        for half, s_ps in ((0, sA), (1, sB)):
                            ccomm = 2 * mb + half
                            j = t_glob * NCOMM + ccomm
                            if not skip_arg:
                                jk = junkp.tile([128, V], F16, tag="jk")
                                nc.vector._custom_dve(
                                    argmax_op, out=jk, in0=s_ps, in1=ncsq_sb,
                                    accum_out=idxf_buf[:, j : j + 1],
                                )
                            if not skip_maxw:
                                jk2 = junkp.tile([128, V], F16, tag="jk")
                                nc.vector._custom_dve(
                                    maxw_op, out=jk2, in0=s_ps, in1=ncsq_sb,
                                    accum_out=m_buf[:, j : j + 1],
                                )
                        if skip_arg and mb == 0 and t_glob == 0:
                            nc.vector.memset(idxf_buf[:, :], 0.0)
                        if skip_maxw and mb == 0 and t_glob == 0:
                            nc.vector.memset(m_buf[:, :], 0.0)

                # ---- per-octet gather pipeline (after odd blocks) ----------
                # octet b = blocks {2b, 2b+1}, t in [8b, 8b+8); list position
                # i = 4096*k + 16*(t*8+c) + q -> row 1024*t + 128*k + 8*q + c.
                # Chunk (k, b): positions [4096*k + 1024*b, +1024) ->
                # idxw slots [256*k + 64*b, +64).
                if do_gather and blk % 2 == 1:
                    ob = blk // 2
                    js = slice(64 * ob, 64 * (ob + 1))
                    nc.vector.tensor_copy(idx16[:, js], idxf_buf[:, js])
                    for kk in range(8):
                        nc.sync.dma_start(
                            idxw[0:16, 256 * kk + 64 * ob : 256 * kk + 64 * (ob + 1)],
                            idx16[16 * kk : 16 * (kk + 1), js],
                        )
                    rep_view = idxw[:, :].rearrange("p (k s) -> p k s", k=8)
                    for g in range(1, 8):
                        nc.sync.dma_start(
                            rep_view[16 * g : 16 * (g + 1), :, 64 * ob : 64 * (ob + 1)],
                            rep_view[0:16, :, 64 * ob : 64 * (ob + 1)],
                        )
                    for kk in range(8):
                        qdst = qoutp.tile([128, 8, C], F32, tag="qd")
                        nc.gpsimd.dma_gather(
                            out_ap=qdst[:, :, :],
                            in_ap=cb_d.ap(),
                            idxs_ap=idxw[:, 256 * kk + 64 * ob : 256 * kk + 64 * (ob + 1)],
                            num_idxs=1024,
                            num_idxs_reg=1024,
                            elem_size=C,
                        )
                        for cc in range(8):
                            src = qdst[16 * cc : 16 * (cc + 1), :, :]
                            dst = q_d.ap().rearrange(
                                "(t k q c) e -> k c q t e", t=32, k=8, q=16, c=8
                            )[kk, cc, :, 8 * ob : 8 * (ob + 1)]
                            nc.sync.dma_start(dst, src)

            # ---- loss partial: sum(L^2) - sum(max w) ------------------------
            msum = constp.tile([128, 1], F32)
            l2sum = constp.tile([128, 1], F32)
            nc.vector.reduce_sum(msum, m_buf, axis=mybir.AxisListType.X)
            nc.vector.reduce_sum(l2sum, l2_buf, axis=mybir.AxisListType.X)
            lvec = constp.tile([128, 1], F32)
            nc.vector.tensor_sub(lvec, l2sum, msum)
            onescol = constp.tile([128, 1], F32)
            nc.gpsimd.memset(onescol, 1.0)
            loss_ps = ps1p.tile([1, 1], F32, tag="one")
            nc.tensor.matmul(loss_ps, lvec, onescol, start=True, stop=True)
            loss_sb = constp.tile([1, 1], F32)
            nc.scalar.copy(loss_sb, loss_ps)
            nc.sync.dma_start(loss_d.ap(), loss_sb)

    nc.compile()
    return nc


_NC_CACHE = None


def _get_nc():
    global _NC_CACHE
    if _NC_CACHE is None:
        _NC_CACHE = build_kernel()
    return _NC_CACHE


def kernel(x, W, b, codebook, _trace=False):
    x = np.ascontiguousarray(np.asarray(x, dtype=np.float32))
    W = np.ascontiguousarray(np.asarray(W, dtype=np.float32))
    b = np.ascontiguousarray(np.asarray(b, dtype=np.float32))
    codebook = np.ascontiguousarray(np.asarray(codebook, dtype=np.float32))

    xt = x.reshape(B * N, H)          # tokens = (b, n) major
    bt = b.reshape(1, H)
    in_maps = []
    for core in range(NCORES):
        in_maps.append(
            {
                "x_d": xt[TOK * core : TOK * (core + 1)],
                "w_d": W,
                "b_d": bt,
                "cb_d": codebook,
            }
        )
    nc = _get_nc()
    # Transient device wedges (NRT_EXEC_UNIT_UNRECOVERABLE etc.) have been
    # observed to clear on a clean retry; the kernel is pure, so retrying the
    # whole dispatch is safe.
    last_err = None
    for attempt in range(3):
        try:
            res = bass_utils.run_bass_kernel_spmd(
                nc, in_maps, core_ids=list(range(NCORES)), trace=_trace
            )
            # materialize results inside the retry loop: async execution
            # surfaces device errors at fetch time.
            qs = [np.asarray(res.results[i]["q_d"]) for i in range(NCORES)]
            partials = [
                float(np.asarray(res.results[i]["loss_d"])[0, 0])
                for i in range(NCORES)
            ]
            break
        except Exception as e:  # noqa: BLE001
            last_err = e
            if attempt == 2:
                raise
            import time as _time

            _time.sleep(2.0)
    comm_output = np.concatenate(qs, axis=0).reshape(B, N, NCOMM, C)
    vq_loss = np.float32(1.25 * sum(partials) / (B * N * NCOMM * C))
    comm_log_probs = np.zeros((B * 1, N), dtype=np.float32)
    if _trace:
        kernel._last_exec_time_ns = res.exec_time_ns
        kernel._last_results = res
    return comm_output, vq_loss, comm_log_probs
